# revision 25
# baseline (speedup 1.0000x reference)
"""Trainium2 Bass kernel for a 4-layer transformer decoder (self-attn +
cross-attn + FFN, post-residual, exact GELU), distributed over 8 NeuronCores.

Sharding: data-parallel over batch (B=4 -> 4 core pairs); within a pair the
target sequence T=1024 is split in half (512 rows per core). Activations are
kept feature-major ("transposed", [D, T_half]) so every projection is a
single matmul chain with no transposes. Cross-core exchange per layer: one
AllGather of the pair's self-attention K/V, plus one AllGather of the pair's
half of the (x-independent) cross-attention K/V, computed and exchanged one
layer ahead so the wire time hides under compute. Each core receives its
half of the cross-KV weights pre-sliced on the host, so the program stays
fully symmetric across cores.

All matmuls run in bf16 with fp32 PSUM accumulation; the residual stream
stays fp32 in SBUF (bf16 shadows feed matmuls). Weight slabs prefetch
per-projection as [128, 1024] bf16 tiles (2KB DMA lines) on the sync DMA
ring; latency-critical transfers (collective staging and attention K/V
loads) ride the scalar-engine HWDGE ring so they never queue behind weight
traffic. Softmax skips max-subtraction; row sums come free from an appended
ones-column on V; normalization uses the fast approximate reciprocal (via
an SBUF bounce - PSUM reads are not supported by custom DVE ops).

Self-contained: hardcodes all shapes; no file I/O, no sibling imports.
"""
import numpy as np
import ml_dtypes

import concourse.bass as bass
import concourse.mybir as mybir
import concourse.tile as tile
from concourse import bacc
from concourse import bass_utils

F32 = mybir.dt.float32
BF16 = mybir.dt.bfloat16
EXP = mybir.ActivationFunctionType.Exp
GELU = mybir.ActivationFunctionType.Gelu

L, D, H, DK, HID = 4, 1024, 16, 64, 4096
B, T, S = 4, 1024, 1024
R = T // 2              # rows (target positions) per core
N_CORES = 8
DC = D // 128            # 8 feature chunks
SC = S // 128            # 8 key chunks
RC = R // 128            # 4 own-row chunks
KV_ELEMS = D * R + R * D        # sa kT flat + v flat per-core shard (bf16)
KHALF = 4 * 128 * S             # ca kT half (4 chunks)
CA_ELEMS = KHALF + S * 512      # ca kT half + ca v half per core (bf16)
RG = [[0, 1], [2, 3], [4, 5], [6, 7]]

_CACHE = {}


def _emit(nc, tc, pools, dram):
    (xp, xbp, ep, wbp, qp, kvp, kwp, ckp, vap, avp, hp, accp,
     minip, minir, minib, vldp) = pools

    def dma(dst, src):
        nc.sync.dma_start(dst, src)

    def dma2(dst, src):
        nc.scalar.dma_start(dst, src)

    def dma3(dst, src):
        nc.gpsimd.dma_start(dst, src)

    def fetch_slabs(w_ap, col0, ncols, kcn=DC):
        """Prefetch [128, ncols] weight slabs for kc chunks (2KB bf16 lines)."""
        slabs = []
        for kc in range(kcn):
            ws = wbp.tile([128, ncols], BF16, tag="wsb")
            dma(ws[:], w_ap[kc * 128:(kc + 1) * 128, col0:col0 + ncols])
            slabs.append(ws)
        return slabs

    def proj_T(w_ap, rhs_tiles, noc, col0, consume, ppool, og=4):
        """Transposed-mode projection: psum[oc] = sum_kc
        W[kc*128:+128, col0+oc*128:+128].T @ rhs[kc]; consume(oc, psum)."""
        kcn = len(rhs_tiles)
        slabs = fetch_slabs(w_ap, col0, noc * 128, kcn)
        for g0 in range(0, noc, og):
            gn = min(og, noc - g0)
            psums = []
            for _ in range(gn):
                ps = ppool.tile([128, R], F32, tag="pj")
                psums.append(ps)
            for kc in range(kcn):
                for j in range(gn):
                    nc.tensor.matmul(
                        psums[j][:],
                        slabs[kc][:, (g0 + j) * 128:(g0 + j + 1) * 128],
                        rhs_tiles[kc][:],
                        start=(kc == 0), stop=(kc == kcn - 1))
            for j in range(gn):
                consume(g0 + j, psums[j])

    def attention(q_tiles, kT_of_wave, va_tiles, spool, avpool):
        """Feature-major attention. Returns DC bf16 [128,R] avT tiles."""
        avT = []
        for _ in range(H // 2):
            at = avp.tile([128, R], BF16, tag="avt")
            avT.append(at)
        for w in range(H // 2):
            kw = kT_of_wave(w)
            avs = []
            for _ in range(2):
                av_ps = avpool.tile([128, R], F32, tag="av")
                avs.append(av_ps)
            for sc in range(SC):
                slab = spool.tile([128, 2 * R], F32, tag="sc")
                p_t = minip.tile([128, 2 * R], BF16, tag="p")
                for hi in range(2):
                    nc.tensor.matmul(
                        slab[:, hi * R:(hi + 1) * R],
                        kw[hi * 64:(hi + 1) * 64, sc * 128:(sc + 1) * 128],
                        q_tiles[w][hi * 64:(hi + 1) * 64, :],
                        start=True, stop=True)
                nc.scalar.activation(p_t[:], slab[:], EXP, scale=0.125)
                for hi in range(2):
                    h = 2 * w + hi
                    nc.tensor.matmul(
                        avs[hi][0:65, :],
                        va_tiles[sc][:, h * 65:(h + 1) * 65],
                        p_t[:, hi * R:(hi + 1) * R],
                        start=(sc == 0), stop=(sc == SC - 1))
            drow = minir.tile([1, 2 * R], F32, tag="drow")
            for hi in range(2):
                nc.vector.tensor_copy(drow[:, hi * R:(hi + 1) * R],
                                      avs[hi][64:65, :])
            rec = minir.tile([1, 2 * R], F32, tag="rec")
            nc.vector.reciprocal_approx_fast(rec[:], drow[:])
            for hi in range(2):
                bc = minib.tile([64, R], F32, tag="bc")
                nc.gpsimd.partition_broadcast(bc[:], rec[:, hi * R:(hi + 1) * R])
                nc.vector.tensor_mul(
                    avT[w][hi * 64:(hi + 1) * 64, :],
                    avs[hi][0:64, :], bc[:])
        return avT

    def phase_kv(li, xT, wqkv, ccin):
        with tc.tile_pool(name=f"psA{li}", bufs=8, space="PSUM") as pA:
            def mk_k(oc, ps):
                kt = kvp.tile([128, R], BF16, tag="ko")
                nc.vector.tensor_copy(kt[:], ps[:])
                dma2(ccin[oc * 128 * R:(oc + 1) * 128 * R]
                     .rearrange("(p s) -> p s", p=128), kt[:])
            proj_T(wqkv, xT, DC, D, mk_k, pA)

            vslabs = fetch_slabs(wqkv, 2 * D, 1024)
            for vc in range(2):
                psv = []
                for _ in range(RC):
                    ps = pA.tile([128, 512], F32, tag="pj")
                    psv.append(ps)
                for kc in range(DC):
                    for t_ in range(RC):
                        nc.tensor.matmul(
                            psv[t_][:], xT[kc][:, t_ * 128:(t_ + 1) * 128],
                            vslabs[kc][:, vc * 512:(vc + 1) * 512],
                            start=(kc == 0), stop=(kc == DC - 1))
                for t_ in range(RC):
                    vt = kvp.tile([128, 512], BF16, tag="vo")
                    nc.vector.tensor_copy(vt[:], psv[t_][:])
                    rbase = D * R + t_ * 128 * D
                    dst = (ccin[rbase:rbase + 128 * D]
                           .rearrange("(p f) -> p f", f=D)
                           [:, vc * 512:(vc + 1) * 512])
                    dma2(dst, vt[:])

    def phase_q(li, xT, wqkv):
        qT = [None] * DC
        with tc.tile_pool(name=f"psQ{li}", bufs=8, space="PSUM") as pQ:
            def mk_q(oc, ps):
                t = qp.tile([128, R], BF16, tag="q")
                nc.vector.tensor_copy(t[:], ps[:])
                qT[oc] = t
            proj_T(wqkv, xT, DC, 0, mk_q, pQ)
        return qT

    def phase_cakv_half(li, encT, wmy, ccain):
        """Compute this core's half of layer li's cross-attn K/V from the
        host-presliced weight and stage it for the ca AllGather."""
        slabs = fetch_slabs(wmy, 0, 1024)
        with (
            tc.tile_pool(name=f"psCk{li}", bufs=2, space="PSUM") as pCk,
            tc.tile_pool(name=f"psCv{li}", bufs=4, space="PSUM") as pCv,
        ):
            for g0 in range(0, 4, 2):
                psums = []
                for _ in range(2):
                    ps = pCk.tile([128, S], F32, tag="pjs")
                    psums.append(ps)
                for kc in range(DC):
                    for sh in range(2):
                        for j in range(2):
                            nc.tensor.matmul(
                                psums[j][:, sh * 512:(sh + 1) * 512],
                                slabs[kc][:, (g0 + j) * 128:(g0 + j + 1) * 128],
                                encT[kc][:, sh * 512:(sh + 1) * 512],
                                start=(kc == 0), stop=(kc == DC - 1))
                for j in range(2):
                    ck = kvp.tile([128, S], BF16, tag="ckst")
                    nc.vector.tensor_copy(ck[:], psums[j][:])
                    dma2(ccain[(g0 + j) * 128 * S:(g0 + j + 1) * 128 * S]
                         .rearrange("(p s) -> p s", p=128), ck[:])
            for sh in range(2):
                psv = []
                for _ in range(4):
                    ps = pCv.tile([128, 512], F32, tag="pj")
                    psv.append(ps)
                for kc in range(DC):
                    for t_ in range(4):
                        sc = sh * 4 + t_
                        nc.tensor.matmul(
                            psv[t_][:],
                            encT[kc][:, sc * 128:(sc + 1) * 128],
                            slabs[kc][:, 512:1024],
                            start=(kc == 0), stop=(kc == DC - 1))
                for t_ in range(4):
                    sc = sh * 4 + t_
                    vt = kvp.tile([128, 512], BF16, tag="vo")
                    nc.vector.tensor_copy(vt[:], psv[t_][:])
                    dma2(ccain[KHALF + sc * 128 * 512:
                               KHALF + (sc + 1) * 128 * 512]
                         .rearrange("(p f) -> p f", f=512), vt[:])

    def phase_cakv_full(encT, wcakv):
        ca_kT = [None] * DC
        ca_va = []
        for sc in range(SC):
            cav = vap.tile([128, H * 65], BF16, tag="cav")
            cav3 = cav[:].rearrange("p (h w) -> p h w", w=65)
            nc.gpsimd.memset(cav3[:, :, 64:65], 1.0)
            ca_va.append(cav)
        kslabs = fetch_slabs(wcakv, 0, 1024)
        vslabs = fetch_slabs(wcakv, D, 1024)
        with (
            tc.tile_pool(name="psCkF", bufs=2, space="PSUM") as pCk,
            tc.tile_pool(name="psCvF", bufs=4, space="PSUM") as pCv,
        ):
            for g0 in range(0, DC, 2):
                psums = []
                for _ in range(2):
                    ps = pCk.tile([128, S], F32, tag="pjs")
                    psums.append(ps)
                for kc in range(DC):
                    for sh in range(2):
                        for j in range(2):
                            nc.tensor.matmul(
                                psums[j][:, sh * 512:(sh + 1) * 512],
                                kslabs[kc][:, (g0 + j) * 128:(g0 + j + 1) * 128],
                                encT[kc][:, sh * 512:(sh + 1) * 512],
                                start=(kc == 0), stop=(kc == DC - 1))
                for j in range(2):
                    ckt = ckp.tile([128, S], BF16, tag="ck")
                    nc.vector.tensor_copy(ckt[:], psums[j][:])
                    ca_kT[g0 + j] = ckt
            for vc in range(2):
                for sh in range(2):
                    psv = []
                    for _ in range(4):
                        ps = pCv.tile([128, 512], F32, tag="pj")
                        psv.append(ps)
                    for kc in range(DC):
                        for t_ in range(4):
                            sc = sh * 4 + t_
                            nc.tensor.matmul(
                                psv[t_][:],
                                encT[kc][:, sc * 128:(sc + 1) * 128],
                                vslabs[kc][:, vc * 512:(vc + 1) * 512],
                                start=(kc == 0), stop=(kc == DC - 1))
                    for t_ in range(4):
                        sc = sh * 4 + t_
                        dst = (ca_va[sc][:]
                               .rearrange("p (h w) -> p h w", w=65)
                               [:, vc * 8:(vc + 1) * 8, 0:DK])
                        srcp = psv[t_][:].rearrange("p (h w) -> p h w", w=DK)
                        nc.vector.tensor_copy(dst, srcp)
        return ca_kT, ca_va

    def phase_ca_load(li, ccaout):
        """Load the gathered cross-attn K/V (both halves) from DRAM."""
        ca_kT = []
        for oc in range(DC):
            rank, idx = oc // 4, oc % 4
            base = rank * CA_ELEMS + idx * 128 * S
            ckt = ckp.tile([128, S], BF16, tag="ck")
            dma3(ckt[:], ccaout[base:base + 128 * S]
                 .rearrange("(p s) -> p s", p=128))
            ca_kT.append(ckt)
        ca_va = []
        for sc in range(SC):
            cav = vap.tile([128, H * 65], BF16, tag="cav")
            cav3 = cav[:].rearrange("p (h w) -> p h w", w=65)
            nc.gpsimd.memset(cav3[:, :, 64:65], 1.0)
            for rank in range(2):
                base = rank * CA_ELEMS + KHALF + sc * 128 * 512
                vload = vldp.tile([128, 512], BF16, tag="vl2")
                dma2(vload[:], ccaout[base:base + 128 * 512]
                     .rearrange("(p f) -> p f", f=512))
                nc.vector.tensor_copy(
                    cav3[:, rank * 8:(rank + 1) * 8, 0:DK],
                    vload[:].rearrange("p (h w) -> p h w", w=DK))
            ca_va.append(cav)
        return ca_kT, ca_va

    def phase_sa_attn(li, qT, ccout):
        sa_va = []
        for sc in range(SC):
            sav = vap.tile([128, H * 65], BF16, tag="sav")
            sav3 = sav[:].rearrange("p (h w) -> p h w", w=65)
            nc.gpsimd.memset(sav3[:, :, 64:65], 1.0)
            blk = sc // 4
            rbase = blk * KV_ELEMS + D * R + (sc % 4) * 128 * D
            vload = vldp.tile([128, D], BF16, tag="vl")
            dma2(vload[:], ccout[rbase:rbase + 128 * D]
                 .rearrange("(p f) -> p f", f=D))
            nc.vector.tensor_copy(
                sav3[:, :, 0:DK],
                vload[:].rearrange("p (h w) -> p h w", w=DK))
            sa_va.append(sav)

        def kT_wave(w):
            kw = kwp.tile([128, S], BF16, tag="kw")
            for blk in range(2):
                base = blk * KV_ELEMS + w * 128 * R
                dma2(kw[:, blk * R:(blk + 1) * R],
                     ccout[base:base + 128 * R]
                     .rearrange("(p s) -> p s", p=128))
            return kw

        with (
            tc.tile_pool(name=f"psD{li}", bufs=2, space="PSUM") as sD,
            tc.tile_pool(name=f"paD{li}", bufs=4, space="PSUM") as aD,
        ):
            return attention(qT, kT_wave, sa_va, sD, aD)

    def phase_proj_res(li, name, w_ap, rhs_tiles, res_tiles, shadow=False):
        """x_out = W.T @ rhs + res; returns new x tiles (+bf16 shadows)."""
        xo = [None] * DC
        xob = [None] * DC
        with tc.tile_pool(name=f"ps{name}{li}", bufs=8, space="PSUM") as pp:
            def mk(oc, ps):
                t = xp.tile([128, R], F32, tag="x")
                nc.vector.tensor_add(t[:], ps[:], res_tiles[oc][:])
                xo[oc] = t
                if shadow:
                    tb = xbp.tile([128, R], BF16, tag="x2b")
                    nc.vector.tensor_copy(tb[:], t[:])
                    xob[oc] = tb
            proj_T(w_ap, rhs_tiles, DC, 0, mk, pp)
        return (xo, xob) if shadow else xo

    def phase_caq(li, wcaq, x1b):
        caqT = [None] * DC
        with tc.tile_pool(name=f"psF{li}", bufs=8, space="PSUM") as pF:
            def mk(oc, ps):
                t = qp.tile([128, R], BF16, tag="q")
                nc.vector.tensor_copy(t[:], ps[:])
                caqT[oc] = t
            proj_T(wcaq, x1b, DC, 0, mk, pF)
        return caqT

    def phase_ca_attn(li, caqT, ca_kT, ca_va):
        with (
            tc.tile_pool(name=f"psG{li}", bufs=2, space="PSUM") as sG,
            tc.tile_pool(name=f"paG{li}", bufs=4, space="PSUM") as aG,
        ):
            return attention(caqT, lambda w: ca_kT[w], ca_va, sG, aG)

    def phase_ffn(li, wf1, wf2, x2, x2b):
        acc = [None] * DC
        x3 = [None] * DC
        x3b = [None] * DC
        with tc.tile_pool(name=f"psI{li}", bufs=8, space="PSUM") as pI:
            for qtr in range(4):
                hq = [None] * DC
                def mk_h(oc, ps, hq=hq):
                    t = hp.tile([128, R], BF16, tag="h")
                    nc.scalar.activation(t[:], ps[:], GELU)
                    hq[oc] = t
                proj_T(wf1, x2b, DC, qtr * D, mk_h, pI)
                wf2q = wf2[qtr * D:(qtr + 1) * D, :]
                def mk_acc(oc, ps, qtr=qtr):
                    if qtr == 0:
                        t = accp.tile([128, R], F32, tag="acc")
                        nc.vector.tensor_add(t[:], ps[:], x2[oc][:])
                        acc[oc] = t
                    elif qtr < 3:
                        nc.vector.tensor_add(acc[oc][:], ps[:], acc[oc][:])
                    else:
                        xt3 = xp.tile([128, R], F32, tag="x")
                        nc.vector.tensor_add(xt3[:], ps[:], acc[oc][:])
                        x3[oc] = xt3
                        xb3 = xbp.tile([128, R], BF16, tag="xb")
                        nc.vector.tensor_copy(xb3[:], xt3[:])
                        x3b[oc] = xb3
                proj_T(wf2q, hq, DC, 0, mk_acc, pI)
        return x3, x3b

    # ---------------- main program ----------------
    (xT_d, xTb_d, encT_d, w_sa_qkv, w_sa_out, w_ca_q, w_ca_kv_my,
     w_ca_kv0, w_ca_out, w_ff1, w_ff2, out_d, cc_in, cc_out, cc_ca_in,
     cc_ca_out) = dram

    xT = []
    xTb = []
    for ci in range(DC):
        xt = xp.tile([128, R], F32, tag="x")
        dma2(xt[:], xT_d.ap()[ci * 128:(ci + 1) * 128])
        xT.append(xt)
        xtb = xbp.tile([128, R], BF16, tag="xb")
        dma2(xtb[:], xTb_d.ap()[ci * 128:(ci + 1) * 128])
        xTb.append(xtb)
    encT = []
    for ci in range(DC):
        et = ep.tile([128, S], BF16, tag="enc")
        dma2(et[:], encT_d.ap()[ci * 128:(ci + 1) * 128])
        encT.append(et)

    def ag(ins_t, outs_t):
        nc.gpsimd.collective_compute(
            "AllGather", mybir.AluOpType.bypass, replica_groups=RG,
            ins=[ins_t], outs=[outs_t])

    for li in range(L):
        ccin = cc_in[li].ap()
        ccout = cc_out[li].ap()
        phase_kv(li, xTb, w_sa_qkv.ap()[li], ccin)
        ag(ccin, ccout)
        qT = phase_q(li, xTb, w_sa_qkv.ap()[li])
        if li == 0:
            ca_kT, ca_va = phase_cakv_full(encT, w_ca_kv0.ap())
        else:
            ca_kT, ca_va = phase_ca_load(li, cc_ca_out[li].ap())
        if li + 1 < L:
            phase_cakv_half(li + 1, encT, w_ca_kv_my.ap()[li + 1],
                            cc_ca_in[li + 1].ap())
            ag(cc_ca_in[li + 1].ap(), cc_ca_out[li + 1].ap())
        avT = phase_sa_attn(li, qT, ccout)
        x1, x1b = phase_proj_res(li, "E", w_sa_out.ap()[li], avT, xT,
                                 shadow=True)
        caqT = phase_caq(li, w_ca_q.ap()[li], x1b)
        ca_avT = phase_ca_attn(li, caqT, ca_kT, ca_va)
        x2, x2b = phase_proj_res(li, "H", w_ca_out.ap()[li], ca_avT, x1,
                                 shadow=True)
        xT, xTb = phase_ffn(li, w_ff1.ap()[li], w_ff2.ap()[li], x2, x2b)

    for oc in range(DC):
        dma(out_d.ap()[oc * 128:(oc + 1) * 128], xT[oc][:])


def _build():
    nc = bacc.Bacc("TRN2", target_bir_lowering=False, debug=False,
                   num_devices=N_CORES)
    dram = (
        nc.dram_tensor("xT", [D, R], F32, kind="ExternalInput"),
        nc.dram_tensor("xTb", [D, R], BF16, kind="ExternalInput"),
        nc.dram_tensor("encT", [D, S], BF16, kind="ExternalInput"),
        nc.dram_tensor("w_sa_qkv", [L, D, 3 * D], BF16, kind="ExternalInput"),
        nc.dram_tensor("w_sa_out", [L, D, D], BF16, kind="ExternalInput"),
        nc.dram_tensor("w_ca_q", [L, D, D], BF16, kind="ExternalInput"),
        nc.dram_tensor("w_ca_kv_my", [L, D, 1024], BF16,
                       kind="ExternalInput"),
        nc.dram_tensor("w_ca_kv0", [D, 2 * D], BF16, kind="ExternalInput"),
        nc.dram_tensor("w_ca_out", [L, D, D], BF16, kind="ExternalInput"),
        nc.dram_tensor("w_ff1", [L, D, HID], BF16, kind="ExternalInput"),
        nc.dram_tensor("w_ff2", [L, HID, D], BF16, kind="ExternalInput"),
        nc.dram_tensor("out", [D, R], F32, kind="ExternalOutput"),
        [nc.dram_tensor(f"cc_in{i}", [KV_ELEMS], BF16, kind="Internal")
         for i in range(L)],
        [nc.dram_tensor(f"cc_out{i}", [2 * KV_ELEMS], BF16, kind="Internal")
         for i in range(L)],
        [nc.dram_tensor(f"cc_ca_in{i}", [CA_ELEMS], BF16, kind="Internal")
         for i in range(L)],
        [nc.dram_tensor(f"cc_ca_out{i}", [2 * CA_ELEMS], BF16,
                        kind="Internal")
         for i in range(L)],
    )
    with tile.TileContext(nc) as tc:
        with (
            tc.tile_pool(name="xp", bufs=12) as xp,      # f32 [128,R] residual
            tc.tile_pool(name="xbp", bufs=8) as xbp,     # bf16 [128,R] shadows
            tc.tile_pool(name="ep", bufs=8) as ep,       # bf16 [128,S] encT
            tc.tile_pool(name="wbp", bufs=10) as wbp,    # bf16 [128,1024] w slabs
            tc.tile_pool(name="qp", bufs=8) as qp,       # bf16 [128,R] qT/caqT
            tc.tile_pool(name="kvp", bufs=3) as kvp,     # bf16 kv staging
            tc.tile_pool(name="kwp", bufs=2) as kwp,     # bf16 [128,S] kT wave
            tc.tile_pool(name="ckp", bufs=8) as ckp,     # bf16 [128,S] ca_kT
            tc.tile_pool(name="vap", bufs=8) as vap,     # bf16 [128,H*65] v_aug
            tc.tile_pool(name="avp", bufs=8) as avp,     # bf16 [128,R] avT
            tc.tile_pool(name="hp", bufs=8) as hp,       # bf16 [128,R] ffn hid
            tc.tile_pool(name="accp", bufs=8) as accp,   # f32 [128,R] ffn acc
            tc.tile_pool(name="minip", bufs=3) as minip,  # bf16 p slabs
            tc.tile_pool(name="minir", bufs=1) as minir,  # drow/rec rows
            tc.tile_pool(name="minib", bufs=2) as minib,
            tc.tile_pool(name="vldp", bufs=2) as vldp,  # bcast tiles
        ):
            pools = (xp, xbp, ep, wbp, qp, kvp, kwp, ckp, vap, avp, hp, accp,
                     minip, minir, minib, vldp)
            _emit(nc, tc, pools, dram)
    nc.compile()
    return nc


def _get_nc():
    if "nc" not in _CACHE:
        _CACHE["nc"] = _build()
    return _CACHE["nc"]


def _prep_in_maps(inputs):
    tgt = np.asarray(inputs["tgt"], dtype=np.float32)
    enc_out = np.asarray(inputs["enc_out"], dtype=np.float32)
    ca_kv_w = np.asarray(inputs["ca_kv_w"], dtype=np.float32)
    shared = {
        "w_sa_qkv": np.asarray(inputs["sa_qkv_w"]).astype(ml_dtypes.bfloat16),
        "w_sa_out": np.asarray(inputs["sa_out_w"]).astype(ml_dtypes.bfloat16),
        "w_ca_q": np.asarray(inputs["ca_q_w"]).astype(ml_dtypes.bfloat16),
        "w_ca_out": np.asarray(inputs["ca_out_w"]).astype(ml_dtypes.bfloat16),
        "w_ff1": np.asarray(inputs["ff_w1"]).astype(ml_dtypes.bfloat16),
        "w_ff2": np.asarray(inputs["ff_w2"]).astype(ml_dtypes.bfloat16),
    }
    ca_kv0 = np.ascontiguousarray(ca_kv_w[0]).astype(ml_dtypes.bfloat16)
    ca_my = [
        np.ascontiguousarray(np.concatenate(
            [ca_kv_w[:, :, hh * 512:(hh + 1) * 512],
             ca_kv_w[:, :, D + hh * 512:D + (hh + 1) * 512]],
            axis=2)).astype(ml_dtypes.bfloat16)
        for hh in range(2)
    ]
    in_maps = []
    for c in range(N_CORES):
        b, hh = c // 2, c % 2
        m = {
            "xT": np.ascontiguousarray(tgt[b].T[:, hh * R:(hh + 1) * R]),
            "xTb": np.ascontiguousarray(
                tgt[b].T[:, hh * R:(hh + 1) * R]).astype(ml_dtypes.bfloat16),
            "encT": np.ascontiguousarray(enc_out[b].T).astype(ml_dtypes.bfloat16),
            "w_ca_kv_my": ca_my[hh],
            "w_ca_kv0": ca_kv0,
        }
        m.update(shared)
        in_maps.append(m)
    return in_maps


def kernel(**inputs):
    nc = _get_nc()
    in_maps = _prep_in_maps(inputs)
    res = bass_utils.run_bass_kernel_spmd(nc, in_maps,
                                          core_ids=list(range(N_CORES)))
    out = np.empty((B, T, D), dtype=np.float32)
    for c in range(N_CORES):
        b, hh = c // 2, c % 2
        out[b, hh * R:(hh + 1) * R, :] = res.results[c]["out"].T
    return out
